# revision 3
# baseline (speedup 1.0000x reference)
"""Trainium2 Bass kernel for nn_MultiHeadAttention_89524298318221.

Full MHA: B=2, S=SQ=2048, d_model=1024, 16 heads (depth 64), fp32.

Sharding (8 cores): data-parallel over batch (2) x tensor-parallel over
head groups (4 heads/core).  Host transposes activations to [d_model, seq]
layout, pre-slices per-head weight columns/rows, and sums the row-parallel
Wo partial products (+ bias) after gathering.

Per-core pipeline (all matmuls in float32r = full PE rate, fp32 PSUM accum):
  1. Project K^T, V (streamed in 4 super-blocks of 512 kpos) and Q^T per
     q-tile.  Layouts: Q^T/K^T as [128 (head-pair x 64 depth), seq], V as
     [128 kpos, ktile, head, 65] with a ones column appended.
  2. Attention per (q-tile 512, head-pair): S^T chunk [128 k, 2x512 q] via
     row-tiled (tile_position) matmul pair, one exp ACTIVATE per chunk over
     both heads (scale=1/8, key-padding mask folded into the per-partition
     bias), then PV with the [V | 1] stationary so row 64 of the PSUM
     accumulator is the softmax denominator.
  3. Normalize via reciprocal + selector-matmul partition broadcast, then
     the output projection Y = O @ Wo_part per q-tile.
"""

import sys

sys.path.insert(0, "/opt/trn_rl_repo")

import numpy as np

import concourse.bass as bass
import concourse.tile as tile
from concourse import bacc, mybir

F32 = mybir.dt.float32
F32R = mybir.dt.float32r
EXP = mybir.ActivationFunctionType.Exp

B, S, DM = 2, 2048, 1024
H, DEPTH = 16, 64
NCORES = 8
HPC = 4            # heads per core
PAIRS = 2          # head pairs per core
HSL = HPC * DEPTH  # 256, per-core head-column slice of d_model
NSB = 4            # k super-blocks of 512
NQT = 4            # q tiles of 512
NCH = 16           # k chunks of 128


def _build():
    nc = bacc.Bacc("TRN2")

    qT = nc.declare_dram_parameter("qT", [DM, S], F32R, isOutput=False)
    kT = nc.declare_dram_parameter("kT", [DM, S], F32R, isOutput=False)
    vT = nc.declare_dram_parameter("vT", [DM, S], F32R, isOutput=False)
    wq = nc.declare_dram_parameter("wq", [DM, HSL], F32R, isOutput=False)
    wk = nc.declare_dram_parameter("wk", [DM, HSL], F32R, isOutput=False)
    wv = nc.declare_dram_parameter("wv", [DM, HSL], F32R, isOutput=False)
    wo = nc.declare_dram_parameter("wo", [HSL, DM], F32R, isOutput=False)
    bqk = nc.declare_dram_parameter("bqk", [128, 4], F32, isOutput=False)
    bv = nc.declare_dram_parameter("bv", [HSL], F32, isOutput=False)
    mb = nc.declare_dram_parameter("mb", [128, NCH], F32, isOutput=False)
    y = nc.declare_dram_parameter("y", [S, DM], F32, isOutput=True)

    with tile.TileContext(nc) as tc:
        with tc.tile_pool(name="single", bufs=1) as sg, \
             tc.tile_pool(name="stream", bufs=1) as st, \
             tc.tile_pool(name="ps", bufs=1, space="PSUM") as ps:

            # ---- persistent SBUF state ----
            wq_sb = sg.tile([128, 8, HSL], F32R)
            nc.sync.dma_start(out=wq_sb, in_=wq.ap().rearrange("(c p) n -> p c n", p=128))
            wk_sb = sg.tile([128, 8, HSL], F32R)
            nc.sync.dma_start(out=wk_sb, in_=wk.ap().rearrange("(c p) n -> p c n", p=128))
            wv_sb = sg.tile([128, 8, HSL], F32R)
            nc.sync.dma_start(out=wv_sb, in_=wv.ap().rearrange("(c p) n -> p c n", p=128))
            wo_sb = sg.tile([128, 2, DM], F32R)
            nc.sync.dma_start(out=wo_sb, in_=wo.ap().rearrange("(j p) n -> p j n", p=128))
            bqk_sb = sg.tile([128, 4], F32)
            nc.sync.dma_start(out=bqk_sb, in_=bqk.ap())
            mb_sb = sg.tile([128, NCH], F32)
            nc.sync.dma_start(out=mb_sb, in_=mb.ap())
            bv_sb = sg.tile([128, HSL], F32)
            nc.sync.dma_start(
                out=bv_sb,
                in_=bass.AP(tensor=bv.ap().tensor, offset=0, ap=[[0, 128], [1, HSL]]),
            )
            ones_col = sg.tile([128, 1], F32)
            nc.vector.memset(ones_col, 1.0)
            # selector weights: broadcast recips row 0 -> out partitions 0-63,
            # row 32 -> partitions 64-127 (f32r matmul dst must start at 0)
            self_f = sg.tile([33, 128], F32)
            nc.vector.memset(self_f, 0.0)
            nc.vector.memset(self_f[0:1, 0:64], 1.0)
            nc.vector.memset(self_f[32:33, 64:128], 1.0)
            sel = sg.tile([33, 128], F32R)
            nc.vector.tensor_copy(out=sel, in_=self_f)
            zer_f = sg.tile([33, 512], F32)
            nc.vector.memset(zer_f, 0.0)
            recips = sg.tile([33, 512], F32R)
            nc.vector.tensor_copy(out=recips, in_=zer_f)

            qt2 = [sg.tile([128, S], F32R, name=f"qt2_{p}") for p in range(PAIRS)]
            kt2 = [sg.tile([128, S], F32R, name=f"kt2_{p}") for p in range(PAIRS)]
            o2t = [sg.tile([128, S], F32R, name=f"o2t_{p}") for p in range(PAIRS)]
            va = sg.tile([128, NCH, HPC, DEPTH + 1], F32R)

            # ---- emission helpers ----
            def emit_kproj(sb_i):
                pk0 = ps.tile([128, 512], F32, tag="aux", bufs=2, name=f"pk0_{sb_i}")
                pk1 = ps.tile([128, 512], F32, tag="aux", bufs=2, name=f"pk1_{sb_i}")
                pk = [pk0, pk1]
                for c in range(8):
                    xk = st.tile([128, 512], F32R, tag="xk", bufs=3, name=f"xk_{sb_i}_{c}")
                    nc.sync.dma_start(
                        out=xk, in_=kT.ap()[c * 128:(c + 1) * 128, sb_i * 512:(sb_i + 1) * 512])
                    for p in range(PAIRS):
                        nc.tensor.matmul(pk[p], wk_sb[:, c, 128 * p:128 * (p + 1)], xk,
                                         start=(c == 0), stop=(c == 7))
                for p in range(PAIRS):
                    nc.vector.tensor_scalar_add(
                        out=kt2[p][:, sb_i * 512:(sb_i + 1) * 512],
                        in0=pk[p], scalar1=bqk_sb[:, 2 + p:3 + p])

            def emit_vproj(sb_i):
                pv0 = ps.tile([128, 2, HSL], F32, tag="aux", bufs=2, name=f"pv0_{sb_i}")
                pv1 = ps.tile([128, 2, HSL], F32, tag="aux", bufs=2, name=f"pv1_{sb_i}")
                pv = [pv0, pv1]
                xvs = []
                for c in range(8):
                    xv = st.tile([128, 512], F32R, tag="xv", bufs=8, name=f"xv_{sb_i}_{c}")
                    nc.sync.dma_start(
                        out=xv, in_=vT.ap()[c * 128:(c + 1) * 128, sb_i * 512:(sb_i + 1) * 512])
                    xvs.append(xv)
                # one accumulation group at a time per PSUM bank: ktiles 0/1
                # sequentially share pv0's bank, 2/3 share pv1's
                for kl in range(4):
                    for c in range(8):
                        nc.tensor.matmul(pv[kl // 2][:, kl % 2, :],
                                         xvs[c][:, kl * 128:(kl + 1) * 128], wv_sb[:, c, :],
                                         start=(c == 0), stop=(c == 7))
                    kt_i = sb_i * 4 + kl
                    nc.vector.tensor_add(
                        out=va[:, kt_i, :, 0:DEPTH],
                        in0=pv[kl // 2][:, kl % 2, :].rearrange("p (h d) -> p h d", h=HPC),
                        in1=bv_sb.rearrange("p (h d) -> p h d", h=HPC))
                    for h in range(HPC):
                        nc.vector.tensor_copy(out=va[:, kt_i, h, DEPTH:DEPTH + 1], in_=ones_col)

            def emit_qproj(qt_i):
                pq0 = ps.tile([128, 512], F32, tag="aux", bufs=2, name=f"pq0_{qt_i}")
                pq1 = ps.tile([128, 512], F32, tag="aux", bufs=2, name=f"pq1_{qt_i}")
                pq = [pq0, pq1]
                for c in range(8):
                    xq = st.tile([128, 512], F32R, tag="xq", bufs=3, name=f"xq_{qt_i}_{c}")
                    nc.sync.dma_start(
                        out=xq, in_=qT.ap()[c * 128:(c + 1) * 128, qt_i * 512:(qt_i + 1) * 512])
                    for p in range(PAIRS):
                        nc.tensor.matmul(pq[p], wq_sb[:, c, 128 * p:128 * (p + 1)], xq,
                                         start=(c == 0), stop=(c == 7))
                for p in range(PAIRS):
                    nc.vector.tensor_scalar_add(
                        out=qt2[p][:, qt_i * 512:(qt_i + 1) * 512],
                        in0=pq[p], scalar1=bqk_sb[:, p:p + 1])

            def emit_attention(qt_i, p, kv_hooks):
                qs = slice(qt_i * 512, (qt_i + 1) * 512)
                o0 = ps.tile([128, 512], F32, tag="o", bufs=2, name=f"o0_{qt_i}_{p}")
                o1 = ps.tile([128, 512], F32, tag="o", bufs=2, name=f"o1_{qt_i}_{p}")
                o_ps = [o0, o1]
                for ch in range(NCH):
                    hook = kv_hooks.get(ch)
                    if hook is not None:
                        hook()
                    cs = slice(ch * 128, (ch + 1) * 128)
                    s_ps = ps.tile([128, 2, 512], F32, tag="s", bufs=2,
                                   name=f"s_{qt_i}_{p}_{ch}")
                    nc.tensor.matmul(s_ps[:, 0, :], kt2[p][0:64, cs], qt2[p][0:64, qs],
                                     start=True, stop=True, tile_position=(0, 0))
                    nc.tensor.matmul(s_ps[:, 1, :], kt2[p][64:128, cs], qt2[p][64:128, qs],
                                     start=True, stop=True, tile_position=(64, 0))
                    es = st.tile([128, 2, 512], F32R, tag="es", bufs=3,
                                 name=f"es_{qt_i}_{p}_{ch}")
                    nc.scalar.activation(out=es, in_=s_ps, func=EXP, scale=0.125,
                                         bias=mb_sb[:, ch:ch + 1])
                    for h in range(2):
                        nc.tensor.matmul(o_ps[h][0:DEPTH + 1, :], va[:, ch, 2 * p + h, :],
                                         es[:, h, :], start=(ch == 0), stop=(ch == NCH - 1))
                # normalization
                with nc.allow_low_precision(reason="f32r reciprocal feeds broadcast matmul"):
                    nc.vector.reciprocal(out=recips[0:1, :], in_=o_ps[0][DEPTH:DEPTH + 1, :])
                    nc.vector.reciprocal(out=recips[32:33, :], in_=o_ps[1][DEPTH:DEPTH + 1, :])
                bc_ps = ps.tile([128, 512], F32, tag="aux", bufs=2, name=f"bc_{qt_i}_{p}")
                nc.tensor.matmul(bc_ps, sel, recips, start=True, stop=True)
                bcs = st.tile([128, 512], F32, tag="bcs", bufs=2, name=f"bcs_{qt_i}_{p}")
                nc.vector.tensor_copy(out=bcs, in_=bc_ps)
                nc.vector.tensor_mul(out=o2t[p][0:64, qs], in0=o_ps[0][0:64, :],
                                     in1=bcs[0:64, :])
                nc.vector.tensor_mul(out=o2t[p][64:128, qs], in0=o_ps[1][0:64, :],
                                     in1=bcs[64:128, :])

            def emit_yproj(qt_i):
                for qsub in range(4):
                    r0 = qt_i * 512 + qsub * 128
                    y_t = st.tile([128, DM], F32, tag="y", bufs=2, name=f"y_{qt_i}_{qsub}")
                    for nt in range(2):
                        yp = ps.tile([128, 512], F32, tag="aux", bufs=2, name=f"yp_{qt_i}_{qsub}_{nt}")
                        for p in range(PAIRS):
                            nc.tensor.matmul(yp, o2t[p][:, r0:r0 + 128],
                                             wo_sb[:, p, nt * 512:(nt + 1) * 512],
                                             start=(p == 0), stop=(p == 1))
                        nc.vector.tensor_copy(out=y_t[:, nt * 512:(nt + 1) * 512], in_=yp)
                    nc.sync.dma_start(out=y.ap()[r0:r0 + 128, :], in_=y_t)

            # ---- emission order ----
            emit_kproj(0)
            emit_vproj(0)
            emit_qproj(0)
            # stream remaining KV projections inside the first attention block
            hooks0 = {
                2: lambda: emit_kproj(1), 4: lambda: emit_vproj(1),
                6: lambda: emit_kproj(2), 8: lambda: emit_vproj(2),
                10: lambda: emit_kproj(3), 12: lambda: emit_vproj(3),
            }
            for qt_i in range(NQT):
                if qt_i > 0:
                    emit_qproj(qt_i)
                for p in range(PAIRS):
                    emit_attention(qt_i, p, hooks0 if (qt_i == 0 and p == 0) else {})
                emit_yproj(qt_i)

    nc.finalize()
    return nc


_STATE = {}


def _runner():
    """Build the kernel once and return a reusable 8-core PJRT callable."""
    if "run" in _STATE:
        return _STATE["run"]

    import jax
    from jax.sharding import Mesh, PartitionSpec
    from jax.experimental.shard_map import shard_map
    from concourse import bass2jax

    nc = _build()
    bass2jax.install_neuronx_cc_hook()

    partition_name = nc.partition_id_tensor.name if nc.partition_id_tensor else None
    in_names, out_names, out_avals, zero_outs = [], [], [], []
    for alloc in nc.m.functions[0].allocations:
        if not isinstance(alloc, mybir.MemoryLocationSet):
            continue
        name = alloc.memorylocations[0].name
        if alloc.kind == "ExternalInput":
            if name != partition_name:
                in_names.append(name)
        elif alloc.kind == "ExternalOutput":
            shape = tuple(alloc.tensor_shape)
            dtype = mybir.dt.np(alloc.dtype)
            out_names.append(name)
            out_avals.append(jax.core.ShapedArray(shape, dtype))
            zero_outs.append(np.zeros(shape, dtype))
    n_params = len(in_names)
    all_in = in_names + out_names
    if partition_name is not None:
        all_in.append(partition_name)

    def _body(*args):
        operands = list(args)
        if partition_name is not None:
            operands.append(bass2jax.partition_id_tensor())
        outs = bass2jax._bass_exec_p.bind(
            *operands,
            out_avals=tuple(out_avals),
            in_names=tuple(all_in),
            out_names=tuple(out_names),
            lowering_input_output_aliases=(),
            sim_require_finite=True,
            sim_require_nnan=True,
            nc=nc,
        )
        return tuple(outs)

    devices = jax.devices()[:NCORES]
    mesh = Mesh(np.asarray(devices), ("core",))
    n_outs = len(out_names)
    sharded = jax.jit(
        shard_map(
            _body, mesh=mesh,
            in_specs=(PartitionSpec("core"),) * (n_params + n_outs),
            out_specs=(PartitionSpec("core"),) * n_outs,
            check_rep=False,
        ),
        keep_unused=True,
    )

    def run(in_maps):
        concat_in = [
            np.concatenate([np.asarray(in_maps[c][nm]) for c in range(NCORES)], axis=0)
            for nm in in_names
        ]
        concat_zeros = [
            np.zeros((NCORES * z.shape[0], *z.shape[1:]), z.dtype) for z in zero_outs
        ]
        out_arrs = sharded(*concat_in, *concat_zeros)
        return [
            {nm: np.asarray(out_arrs[i]).reshape(NCORES, *out_avals[i].shape)[c]
             for i, nm in enumerate(out_names)}
            for c in range(NCORES)
        ]

    _STATE["run"] = run
    _STATE["in_names"] = in_names
    return run


def _make_in_maps(value, key, query, key_padding_mask, Wq, bq, Wk, bk, Wv, bv, Wo):
    f32 = np.float32
    mask_bias = np.where(np.asarray(key_padding_mask), 0.0, -1e30).astype(f32)  # [B, S]
    in_maps = []
    for core in range(NCORES):
        b = core // 4
        g = core % 4
        hs = slice(g * HSL, (g + 1) * HSL)
        bq_c = np.asarray(bq[hs], f32)
        bk_c = np.asarray(bk[hs], f32)
        bqk_c = np.stack([bq_c[0:128], bq_c[128:256], bk_c[0:128], bk_c[128:256]], axis=1)
        in_maps.append({
            "qT": np.ascontiguousarray(np.asarray(query[b], f32).T),
            "kT": np.ascontiguousarray(np.asarray(key[b], f32).T),
            "vT": np.ascontiguousarray(np.asarray(value[b], f32).T),
            "wq": np.ascontiguousarray(np.asarray(Wq, f32)[:, hs]),
            "wk": np.ascontiguousarray(np.asarray(Wk, f32)[:, hs]),
            "wv": np.ascontiguousarray(np.asarray(Wv, f32)[:, hs]),
            "wo": np.ascontiguousarray(np.asarray(Wo, f32)[hs, :]),
            "bqk": np.ascontiguousarray(bqk_c),
            "bv": np.ascontiguousarray(np.asarray(bv[hs], f32)),
            "mb": np.ascontiguousarray(mask_bias[b].reshape(NCH, 128).T),
        })
    return in_maps


def kernel(value, key, query, key_padding_mask, Wq, bq, Wk, bk, Wv, bv, Wo, bo):
    run = _runner()
    in_maps = _make_in_maps(value, key, query, key_padding_mask,
                            Wq, bq, Wk, bk, Wv, bv, Wo)
    results = run(in_maps)
    out = np.zeros((B, S, DM), np.float32)
    for core in range(NCORES):
        out[core // 4] += results[core]["y"]
    out += np.asarray(bo, np.float32)
    return out


# revision 4
# speedup vs baseline: 3005.4739x; 3005.4739x over previous
"""Trainium2 Bass kernel for nn_MultiHeadAttention_89524298318221.

Full MHA: B=2, S=SQ=2048, d_model=1024, 16 heads (depth 64), fp32.

Sharding (8 cores): data-parallel over batch (2) x tensor-parallel over
head groups (4 heads/core).  Host transposes activations to [d_model, seq]
layout, pre-slices per-head weight columns/rows, and sums the row-parallel
Wo partial products (+ bias) after gathering.

Per-core pipeline (all matmuls in float32r = full PE rate, fp32 PSUM accum):
  1. Project K^T, V (streamed in 4 super-blocks of 512 kpos) and Q^T per
     q-tile.  Layouts: Q^T/K^T as [128 (head-pair x 64 depth), seq], V as
     [128 kpos, ktile, head, 65] with a ones column appended.
  2. Attention per (q-tile 512, head-pair): S^T chunk [128 k, 2x512 q] via
     row-tiled (tile_position) matmul pair, one exp ACTIVATE per chunk over
     both heads (scale=1/8, key-padding mask folded into the per-partition
     bias), then PV with the [V | 1] stationary so row 64 of the PSUM
     accumulator is the softmax denominator.
  3. Normalize via reciprocal + selector-matmul partition broadcast, then
     the output projection Y = O @ Wo_part per q-tile.
"""

import sys

sys.path.insert(0, "/opt/trn_rl_repo")

import numpy as np

import concourse.bass as bass
import concourse.tile as tile
from concourse import bacc, mybir

F32 = mybir.dt.float32
F32R = mybir.dt.float32r
EXP = mybir.ActivationFunctionType.Exp

B, S, DM = 2, 2048, 1024
H, DEPTH = 16, 64
NCORES = 8
HPC = 4            # heads per core
PAIRS = 2          # head pairs per core
HSL = HPC * DEPTH  # 256, per-core head-column slice of d_model
NSB = 4            # k super-blocks of 512
NQT = 4            # q tiles of 512
NCH = 16           # k chunks of 128


def _build():
    nc = bacc.Bacc("TRN2")

    qT = nc.declare_dram_parameter("qT", [DM, S], F32R, isOutput=False)
    kT = nc.declare_dram_parameter("kT", [DM, S], F32R, isOutput=False)
    vT = nc.declare_dram_parameter("vT", [DM, S], F32R, isOutput=False)
    wq = nc.declare_dram_parameter("wq", [DM, HSL], F32R, isOutput=False)
    wk = nc.declare_dram_parameter("wk", [DM, HSL], F32R, isOutput=False)
    wv = nc.declare_dram_parameter("wv", [DM, HSL], F32R, isOutput=False)
    wo = nc.declare_dram_parameter("wo", [HSL, DM], F32R, isOutput=False)
    bqk = nc.declare_dram_parameter("bqk", [128, 4], F32, isOutput=False)
    bv = nc.declare_dram_parameter("bv", [HSL], F32, isOutput=False)
    mb = nc.declare_dram_parameter("mb", [128, NCH], F32, isOutput=False)
    y = nc.declare_dram_parameter("y", [S, DM], F32, isOutput=True)

    with tile.TileContext(nc) as tc:
        with tc.tile_pool(name="single", bufs=1) as sg, \
             tc.tile_pool(name="stream", bufs=1) as st, \
             tc.tile_pool(name="ps", bufs=1, space="PSUM") as ps:

            # ---- persistent SBUF state ----
            wq_sb = sg.tile([128, 8, HSL], F32R)
            nc.sync.dma_start(out=wq_sb, in_=wq.ap().rearrange("(c p) n -> p c n", p=128))
            wk_sb = sg.tile([128, 8, HSL], F32R)
            nc.sync.dma_start(out=wk_sb, in_=wk.ap().rearrange("(c p) n -> p c n", p=128))
            wv_sb = sg.tile([128, 8, HSL], F32R)
            nc.sync.dma_start(out=wv_sb, in_=wv.ap().rearrange("(c p) n -> p c n", p=128))
            wo_sb = sg.tile([128, 2, DM], F32R)
            nc.sync.dma_start(out=wo_sb, in_=wo.ap().rearrange("(j p) n -> p j n", p=128))
            bqk_sb = sg.tile([128, 4], F32)
            nc.sync.dma_start(out=bqk_sb, in_=bqk.ap())
            mb_sb = sg.tile([128, NCH], F32)
            nc.sync.dma_start(out=mb_sb, in_=mb.ap())
            bv_sb = sg.tile([128, HSL], F32)
            nc.sync.dma_start(
                out=bv_sb,
                in_=bass.AP(tensor=bv.ap().tensor, offset=0, ap=[[0, 128], [1, HSL]]),
            )
            ones_col = sg.tile([128, 1], F32)
            nc.vector.memset(ones_col, 1.0)
            # selector weights: broadcast recips row 0 -> out partitions 0-63,
            # row 32 -> partitions 64-127 (f32r matmul dst must start at 0)
            self_f = sg.tile([33, 128], F32)
            nc.vector.memset(self_f, 0.0)
            nc.vector.memset(self_f[0:1, 0:64], 1.0)
            nc.vector.memset(self_f[32:33, 64:128], 1.0)
            sel = sg.tile([33, 128], F32R)
            nc.vector.tensor_copy(out=sel, in_=self_f)
            zer_f = sg.tile([33, 512], F32)
            nc.vector.memset(zer_f, 0.0)
            recips = sg.tile([33, 512], F32R)
            nc.vector.tensor_copy(out=recips, in_=zer_f)

            qt2 = [sg.tile([128, S], F32R, name=f"qt2_{p}") for p in range(PAIRS)]
            kt2 = [sg.tile([128, S], F32R, name=f"kt2_{p}") for p in range(PAIRS)]
            o2t = [sg.tile([128, S], F32R, name=f"o2t_{p}") for p in range(PAIRS)]
            va = sg.tile([128, NCH, HPC, DEPTH + 1], F32R)

            # ---- emission helpers ----
            def emit_kproj(sb_i):
                pk0 = ps.tile([128, 512], F32, tag="aux", bufs=2, name=f"pk0_{sb_i}")
                pk1 = ps.tile([128, 512], F32, tag="aux", bufs=2, name=f"pk1_{sb_i}")
                pk = [pk0, pk1]
                for c in range(8):
                    xk = st.tile([128, 512], F32R, tag="xk", bufs=3, name=f"xk_{sb_i}_{c}")
                    nc.sync.dma_start(
                        out=xk, in_=kT.ap()[c * 128:(c + 1) * 128, sb_i * 512:(sb_i + 1) * 512])
                    for p in range(PAIRS):
                        nc.tensor.matmul(pk[p], wk_sb[:, c, 128 * p:128 * (p + 1)], xk,
                                         start=(c == 0), stop=(c == 7))
                for p in range(PAIRS):
                    nc.vector.tensor_scalar_add(
                        out=kt2[p][:, sb_i * 512:(sb_i + 1) * 512],
                        in0=pk[p], scalar1=bqk_sb[:, 2 + p:3 + p])

            def emit_vproj(sb_i):
                pv0 = ps.tile([128, 2, HSL], F32, tag="aux", bufs=2, name=f"pv0_{sb_i}")
                pv1 = ps.tile([128, 2, HSL], F32, tag="aux", bufs=2, name=f"pv1_{sb_i}")
                pv = [pv0, pv1]
                xvs = []
                for c in range(8):
                    xv = st.tile([128, 512], F32R, tag="xv", bufs=8, name=f"xv_{sb_i}_{c}")
                    nc.sync.dma_start(
                        out=xv, in_=vT.ap()[c * 128:(c + 1) * 128, sb_i * 512:(sb_i + 1) * 512])
                    xvs.append(xv)
                # one accumulation group at a time per PSUM bank: ktiles 0/1
                # sequentially share pv0's bank, 2/3 share pv1's
                for kl in range(4):
                    for c in range(8):
                        nc.tensor.matmul(pv[kl // 2][:, kl % 2, :],
                                         xvs[c][:, kl * 128:(kl + 1) * 128], wv_sb[:, c, :],
                                         start=(c == 0), stop=(c == 7))
                    kt_i = sb_i * 4 + kl
                    nc.vector.tensor_add(
                        out=va[:, kt_i, :, 0:DEPTH],
                        in0=pv[kl // 2][:, kl % 2, :].rearrange("p (h d) -> p h d", h=HPC),
                        in1=bv_sb.rearrange("p (h d) -> p h d", h=HPC))
                    for h in range(HPC):
                        nc.vector.tensor_copy(out=va[:, kt_i, h, DEPTH:DEPTH + 1], in_=ones_col)

            def emit_qproj(qt_i):
                pq0 = ps.tile([128, 512], F32, tag="aux", bufs=2, name=f"pq0_{qt_i}")
                pq1 = ps.tile([128, 512], F32, tag="aux", bufs=2, name=f"pq1_{qt_i}")
                pq = [pq0, pq1]
                for c in range(8):
                    xq = st.tile([128, 512], F32R, tag="xq", bufs=3, name=f"xq_{qt_i}_{c}")
                    nc.sync.dma_start(
                        out=xq, in_=qT.ap()[c * 128:(c + 1) * 128, qt_i * 512:(qt_i + 1) * 512])
                    for p in range(PAIRS):
                        nc.tensor.matmul(pq[p], wq_sb[:, c, 128 * p:128 * (p + 1)], xq,
                                         start=(c == 0), stop=(c == 7))
                for p in range(PAIRS):
                    nc.vector.tensor_scalar_add(
                        out=qt2[p][:, qt_i * 512:(qt_i + 1) * 512],
                        in0=pq[p], scalar1=bqk_sb[:, p:p + 1])

            def emit_attention(qt_i, p, kv_hooks):
                qs = slice(qt_i * 512, (qt_i + 1) * 512)
                o0 = ps.tile([128, 512], F32, tag="o", bufs=2, name=f"o0_{qt_i}_{p}")
                o1 = ps.tile([128, 512], F32, tag="o", bufs=2, name=f"o1_{qt_i}_{p}")
                o_ps = [o0, o1]
                for ch in range(NCH):
                    hook = kv_hooks.get(ch)
                    if hook is not None:
                        hook()
                    cs = slice(ch * 128, (ch + 1) * 128)
                    s_ps = ps.tile([128, 2, 512], F32, tag="s", bufs=2,
                                   name=f"s_{qt_i}_{p}_{ch}")
                    nc.tensor.matmul(s_ps[:, 0, :], kt2[p][0:64, cs], qt2[p][0:64, qs],
                                     start=True, stop=True, tile_position=(0, 0))
                    nc.tensor.matmul(s_ps[:, 1, :], kt2[p][64:128, cs], qt2[p][64:128, qs],
                                     start=True, stop=True, tile_position=(64, 0))
                    es = st.tile([128, 2, 512], F32R, tag="es", bufs=3,
                                 name=f"es_{qt_i}_{p}_{ch}")
                    nc.scalar.activation(out=es, in_=s_ps, func=EXP, scale=0.125,
                                         bias=mb_sb[:, ch:ch + 1])
                    for h in range(2):
                        nc.tensor.matmul(o_ps[h][0:DEPTH + 1, :], va[:, ch, 2 * p + h, :],
                                         es[:, h, :], start=(ch == 0), stop=(ch == NCH - 1))
                # normalization
                with nc.allow_low_precision(reason="f32r reciprocal feeds broadcast matmul"):
                    nc.vector.reciprocal(out=recips[0:1, :], in_=o_ps[0][DEPTH:DEPTH + 1, :])
                    nc.vector.reciprocal(out=recips[32:33, :], in_=o_ps[1][DEPTH:DEPTH + 1, :])
                bc_ps = ps.tile([128, 512], F32, tag="aux", bufs=2, name=f"bc_{qt_i}_{p}")
                nc.tensor.matmul(bc_ps, sel, recips, start=True, stop=True)
                bcs = st.tile([128, 512], F32, tag="bcs", bufs=2, name=f"bcs_{qt_i}_{p}")
                nc.vector.tensor_copy(out=bcs, in_=bc_ps)
                nc.vector.tensor_mul(out=o2t[p][0:64, qs], in0=o_ps[0][0:64, :],
                                     in1=bcs[0:64, :])
                nc.vector.tensor_mul(out=o2t[p][64:128, qs], in0=o_ps[1][0:64, :],
                                     in1=bcs[64:128, :])

            def emit_yproj(qt_i):
                for qsub in range(4):
                    r0 = qt_i * 512 + qsub * 128
                    y_t = st.tile([128, DM], F32, tag="y", bufs=2, name=f"y_{qt_i}_{qsub}")
                    for nt in range(2):
                        yp = ps.tile([128, 512], F32, tag="aux", bufs=2, name=f"yp_{qt_i}_{qsub}_{nt}")
                        for p in range(PAIRS):
                            nc.tensor.matmul(yp, o2t[p][:, r0:r0 + 128],
                                             wo_sb[:, p, nt * 512:(nt + 1) * 512],
                                             start=(p == 0), stop=(p == 1))
                        nc.vector.tensor_copy(out=y_t[:, nt * 512:(nt + 1) * 512], in_=yp)
                    nc.sync.dma_start(out=y.ap()[r0:r0 + 128, :], in_=y_t)

            # ---- emission order ----
            emit_kproj(0)
            emit_vproj(0)
            emit_qproj(0)
            # stream remaining KV projections inside the first attention block
            hooks0 = {
                2: lambda: emit_kproj(1), 4: lambda: emit_vproj(1),
                6: lambda: emit_kproj(2), 8: lambda: emit_vproj(2),
                10: lambda: emit_kproj(3), 12: lambda: emit_vproj(3),
            }
            for qt_i in range(NQT):
                if qt_i > 0:
                    emit_qproj(qt_i)
                for p in range(PAIRS):
                    emit_attention(qt_i, p, hooks0 if (qt_i == 0 and p == 0) else {})
                emit_yproj(qt_i)

    nc.finalize()
    return nc


_STATE = {}


def _runner():
    """Build the kernel once and return a reusable 8-core PJRT callable."""
    if "run" in _STATE:
        return _STATE["run"]

    import jax
    from jax.sharding import Mesh, PartitionSpec
    from jax.experimental.shard_map import shard_map
    from concourse import bass2jax

    nc = _build()
    bass2jax.install_neuronx_cc_hook()

    partition_name = nc.partition_id_tensor.name if nc.partition_id_tensor else None
    in_names, out_names, out_avals, zero_outs = [], [], [], []
    for alloc in nc.m.functions[0].allocations:
        if not isinstance(alloc, mybir.MemoryLocationSet):
            continue
        name = alloc.memorylocations[0].name
        if alloc.kind == "ExternalInput":
            if name != partition_name:
                in_names.append(name)
        elif alloc.kind == "ExternalOutput":
            shape = tuple(alloc.tensor_shape)
            dtype = mybir.dt.np(alloc.dtype)
            out_names.append(name)
            out_avals.append(jax.core.ShapedArray(shape, dtype))
            zero_outs.append(np.zeros(shape, dtype))
    n_params = len(in_names)
    all_in = in_names + out_names
    if partition_name is not None:
        all_in.append(partition_name)

    def _body(*args):
        operands = list(args)
        if partition_name is not None:
            operands.append(bass2jax.partition_id_tensor())
        outs = bass2jax._bass_exec_p.bind(
            *operands,
            out_avals=tuple(out_avals),
            in_names=tuple(all_in),
            out_names=tuple(out_names),
            lowering_input_output_aliases=(),
            sim_require_finite=True,
            sim_require_nnan=True,
            nc=nc,
        )
        return tuple(outs)

    devices = jax.devices()[:NCORES]
    mesh = Mesh(np.asarray(devices), ("core",))
    n_outs = len(out_names)
    sharded = jax.jit(
        shard_map(
            _body, mesh=mesh,
            in_specs=(PartitionSpec("core"),) * (n_params + n_outs),
            out_specs=(PartitionSpec("core"),) * n_outs,
            check_rep=False,
        ),
        keep_unused=True,
    )

    def run(in_maps):
        concat_in = [
            np.concatenate([np.asarray(in_maps[c][nm]) for c in range(NCORES)], axis=0)
            for nm in in_names
        ]
        concat_zeros = [
            np.zeros((NCORES * z.shape[0], *z.shape[1:]), z.dtype) for z in zero_outs
        ]
        out_arrs = sharded(*concat_in, *concat_zeros)
        return [
            {nm: np.asarray(out_arrs[i]).reshape(NCORES, *out_avals[i].shape)[c]
             for i, nm in enumerate(out_names)}
            for c in range(NCORES)
        ]

    _STATE["run"] = run
    _STATE["in_names"] = in_names
    _STATE["sharded"] = sharded
    _STATE["mesh"] = mesh
    _STATE["out_names"] = out_names
    _STATE["out_avals"] = out_avals
    _STATE["zero_outs"] = zero_outs
    return run


def _make_in_maps(value, key, query, key_padding_mask, Wq, bq, Wk, bk, Wv, bv, Wo):
    f32 = np.float32
    mask_bias = np.where(np.asarray(key_padding_mask), 0.0, -1e30).astype(f32)  # [B, S]
    in_maps = []
    for core in range(NCORES):
        b = core // 4
        g = core % 4
        hs = slice(g * HSL, (g + 1) * HSL)
        bq_c = np.asarray(bq[hs], f32)
        bk_c = np.asarray(bk[hs], f32)
        bqk_c = np.stack([bq_c[0:128], bq_c[128:256], bk_c[0:128], bk_c[128:256]], axis=1)
        in_maps.append({
            "qT": np.ascontiguousarray(np.asarray(query[b], f32).T),
            "kT": np.ascontiguousarray(np.asarray(key[b], f32).T),
            "vT": np.ascontiguousarray(np.asarray(value[b], f32).T),
            "wq": np.ascontiguousarray(np.asarray(Wq, f32)[:, hs]),
            "wk": np.ascontiguousarray(np.asarray(Wk, f32)[:, hs]),
            "wv": np.ascontiguousarray(np.asarray(Wv, f32)[:, hs]),
            "wo": np.ascontiguousarray(np.asarray(Wo, f32)[hs, :]),
            "bqk": np.ascontiguousarray(bqk_c),
            "bv": np.ascontiguousarray(np.asarray(bv[hs], f32)),
            "mb": np.ascontiguousarray(mask_bias[b].reshape(NCH, 128).T),
        })
    return in_maps


def kernel(value, key, query, key_padding_mask, Wq, bq, Wk, bk, Wv, bv, Wo, bo):
    run = _runner()
    in_maps = _make_in_maps(value, key, query, key_padding_mask,
                            Wq, bq, Wk, bk, Wv, bv, Wo)
    results = run(in_maps)
    out = np.zeros((B, S, DM), np.float32)
    for core in range(NCORES):
        out[core // 4] += results[core]["y"]
    out += np.asarray(bo, np.float32)
    return out
